# revision 16
# baseline (speedup 1.0000x reference)
"""Trainium2 Bass kernel for nn_Block_420906795461 (dense transformer block).

Data-parallel over B=8 clip-pairs across 8 NeuronCores; each core runs the
full block on its [2, 512, 2048] shard with replicated weights; no
collectives.

v2: fp8e4 DoubleRow (2 MACs/PE/cycle) on the precision-safe matmuls —
LN1 stats, QKV (q, k, v) and the attention output projection — with
host-side fp8 weight planes and fp8 activations (LN1 output, attention
output). Attention scores / softmax / AV and the whole MLP stay bf16:
numpy simulation showed e4m3 noise there breaks the 2e-2 gate (exp
amplifies score errors; quantized exp weights flip near-tie attention
mixes; MLP output noise 2.5%+ of a branch that carries half the output
variance). Measured config error (sim): rel ~9e-3 vs gate 2e-2.

Scale carrying: wq/wk/wp prescaled x32, wv x16 (host). q,k bf16 carry
x32 -> exp scale = SCALE/1024. v carries x16 -> attention out fp8
carries x16 -> proj psum carries x512, divided out at the residual add.
bproj is folded into the residual input xT host-side.

Per-core dataflow (activations feature-major [feature, token] in SBUF,
tokens 0..511 = clip0, 512..1023 = clip1):
  LN1 (fp8 x, DR stats) -> QKV DR-fp8 -> attention bf16 (per-head,
  clips interleaved; out -> fp8 o_dr in SBUF) -> proj DR-fp8 + residual
  -> LN2 (fp32r stats) -> MLP bf16 (W1 + erf-gelu + W2, PSUM chunks
  added in place into the fp32 residual tiles).

LayerNorm stats and softmax denominators use all-ones stationary
matmuls (partition reduction pre-broadcast). Softmax skips
max-subtraction (scores ~N(0,1)); exp scale and the denominator divide
fold into PSUM-evacuation ops. rstd via exp(-0.5*ln(var+eps)) keeps one
ACT table pre-MLP.
"""

import sys

import numpy as np

sys.path.insert(0, "/opt/trn_rl_repo")

from contextlib import ExitStack

import concourse.bass as bass  # noqa: F401
import concourse.mybir as mybir
import concourse.tile as tile
from concourse import bacc
from concourse.bass_utils import run_bass_kernel_spmd

FP32 = mybir.dt.float32
FP32R = mybir.dt.float32r
BF16 = mybir.dt.bfloat16
FP8 = mybir.dt.float8e4
AF = mybir.ActivationFunctionType
ALU = mybir.AluOpType
DR = mybir.MatmulPerfMode.DoubleRow

DIM = 2048
HEADS = 16
HD = 128
F = 4 * DIM          # 8192
TOK = 1024           # tokens per core (2 clips x 512)
NH = 512             # tokens per clip
CT = DIM // 128      # 16 c-tiles
NT = CT // 2         # 8 DoubleRow k-steps over c
FT = F // 128        # 64 f-tiles
CH = 16              # mlp chunk size in f-tiles
SCALE = HD ** -0.5
EPS = 1e-5
N_CORES = 8
WS = 32.0            # fp8 prescale for wq/wk/wp
VS = 16.0            # fp8 prescale for wv (v + attn out carry x16)


def _patch_act_tables():
    """Force the ACT-table chooser onto two sets that jointly cover every
    activation function this kernel uses ({Square, Ln, Exp, Identity, Copy}
    pre-MLP, + Gelu), so the table is swapped once instead of ~6 times
    (each mid-kernel ACT_TABLE_LOAD is a 1.28us stall in a serial chain).
    Set ids are positional, so unwanted sets are blanked rather than
    removed."""
    import concourse.bacc as _bacc

    _orig = _bacc.get_activation_tables
    keep = {"natural_log_exp_and_others", "gelu_and_others"}

    def patched(arch):
        return {name: (funcs if name in keep else set())
                for name, funcs in _orig(arch).items()}

    _bacc.get_activation_tables = patched
    return _orig


def build():
    nc = bacc.Bacc("TRN2", target_bir_lowering=False, debug=False)

    xT = nc.dram_tensor("xT", [DIM, TOK], FP32, kind="ExternalInput").ap()
    x8d = nc.dram_tensor("x8", [128, CT, TOK], FP8,
                         kind="ExternalInput").ap()
    wq8d = nc.dram_tensor("wq8", [128, 4 * CT * 512], FP8,
                          kind="ExternalInput").ap()
    wk8d = nc.dram_tensor("wk8", [128, 4 * CT * 512], FP8,
                          kind="ExternalInput").ap()
    wv8d = nc.dram_tensor("wv8", [128, 4 * CT * 512], FP8,
                          kind="ExternalInput").ap()
    wp8d = nc.dram_tensor("wp8", [128, 4 * CT * 512], FP8,
                          kind="ExternalInput").ap()
    w1T = nc.dram_tensor("w1T", [DIM, F], BF16, kind="ExternalInput").ap()
    w2T = nc.dram_tensor("w2T", [F, DIM], BF16, kind="ExternalInput").ap()
    g1v = nc.dram_tensor("g1v", [128, CT], FP32, kind="ExternalInput").ap()
    be1v = nc.dram_tensor("be1v", [128, CT], FP32, kind="ExternalInput").ap()
    g2v = nc.dram_tensor("g2v", [128, CT], FP32, kind="ExternalInput").ap()
    be2v = nc.dram_tensor("be2v", [128, CT], FP32, kind="ExternalInput").ap()
    b1v = nc.dram_tensor("b1v", [128, FT], FP32, kind="ExternalInput").ap()
    b2v = nc.dram_tensor("b2v", [128, CT], FP32, kind="ExternalInput").ap()
    outT = nc.dram_tensor("out", [DIM, TOK], FP32, kind="ExternalOutput").ap()

    with tile.TileContext(nc, pool_alloc_mode="stack") as tc, \
            ExitStack() as top:
        consts = top.enter_context(tc.tile_pool(name="consts", bufs=1))

        onesm_bf = consts.tile([128, 128], BF16, tag="onesmbf")
        nc.vector.memset(onesm_bf, 1.0)
        onesm_f = consts.tile([128, 128], FP32, tag="onesmf")
        nc.vector.memset(onesm_f, 1.0)
        onesm_r = onesm_f.bitcast(FP32R)
        ones8 = consts.tile([128, 2, 128], FP8, tag="ones8")
        nc.vector.memset(ones8, 1.0)
        eps128 = consts.tile([128, 1], FP32, tag="eps")
        nc.vector.memset(eps128, EPS)
        # dummy ACT op: triggers the (single) pre-MLP ACT table load during
        # the initial input DMAs instead of stalling LN1's first Square
        warm = consts.tile([128, 1], FP32, tag="warm")
        nc.scalar.activation(out=warm, in_=eps128, func=AF.Exp)

        def load_const(name, src, cols):
            t = consts.tile([128, cols], FP32, tag=name)
            nc.sync.dma_start(out=t, in_=src)
            return t

        g1s = load_const("g1s", g1v, CT)
        be1s = load_const("be1s", be1v, CT)
        g2s = load_const("g2s", g2v, CT)
        be2s = load_const("be2s", be2v, CT)
        b1s = load_const("b1s", b1v, FT)
        b2s = load_const("b2s", b2v, CT)

        def layernorm_wide(uid, loader, g_s, be_s, out_pool, out_tag,
                           pools):
            """Full-width (TOK) LN over fp32r input tiles (LN2).
            loader(ct) returns a [128, TOK] fp32r AP. Returns 16 bf16
            tiles. pools=(ln_ps, sqp, tmpp, vecp) reuses pre-opened pools
            so the stats can overlap a preceding phase."""
            ln_ps, sqp, tmpp, vecp = pools
            ps_sum = [ln_ps.tile([128, NH], FP32, tag="pj",
                                 name=f"lns{uid}{i}") for i in range(2)]
            ps_sq = [ln_ps.tile([128, NH], FP32, tag="pj",
                                name=f"lnq{uid}{i}") for i in range(2)]
            for ct in range(CT):
                xin = loader(ct)
                sq = sqp.tile([128, TOK], FP32R, tag="sq")
                if ct % 2 == 0:
                    nc.scalar.activation(out=sq, in_=xin.bitcast(FP32),
                                         func=AF.Square)
                else:
                    nc.gpsimd.tensor_mul(out=sq,
                                         in0=xin.bitcast(FP32),
                                         in1=xin.bitcast(FP32))
                for i in range(2):
                    sl = slice(i * NH, (i + 1) * NH)
                    nc.tensor.matmul(out=ps_sum[i], lhsT=onesm_r,
                                     rhs=xin[:, sl],
                                     start=(ct == 0), stop=(ct == CT - 1))
                    nc.tensor.matmul(out=ps_sq[i], lhsT=onesm_r,
                                     rhs=sq[:, sl],
                                     start=(ct == 0), stop=(ct == CT - 1))
            mean_b = vecp.tile([128, TOK], FP32, tag="v")
            ex2 = vecp.tile([128, TOK], FP32, tag="v")
            for i in range(2):
                sl = slice(i * NH, (i + 1) * NH)
                nc.vector.tensor_scalar_mul(out=mean_b[:, sl],
                                            in0=ps_sum[i],
                                            scalar1=1.0 / DIM)
                nc.vector.tensor_scalar_mul(out=ex2[:, sl],
                                            in0=ps_sq[i],
                                            scalar1=1.0 / DIM)
            msq = vecp.tile([128, TOK], FP32, tag="v")
            nc.vector.tensor_mul(out=msq, in0=mean_b, in1=mean_b)
            var = vecp.tile([128, TOK], FP32, tag="v")
            nc.vector.tensor_sub(out=var, in0=ex2, in1=msq)
            # rstd = (var+eps)^-0.5 = exp(-0.5*ln(var+eps)); keeps the
            # whole pre-MLP kernel on one ACT table (no Sqrt swap-ins)
            lnv = vecp.tile([128, TOK], FP32, tag="v")
            nc.scalar.activation(out=lnv, in_=var, func=AF.Ln,
                                 bias=eps128)
            rstd_b = vecp.tile([128, TOK], FP32, tag="v")
            nc.scalar.activation(out=rstd_b, in_=lnv, func=AF.Exp,
                                 scale=-0.5)
            outs = []
            for ct in range(CT):
                xin = loader(ct)
                e_sub = nc.vector if ct % 2 == 0 else nc.gpsimd
                e_mul = nc.gpsimd if ct % 2 == 0 else nc.vector
                t1 = tmpp.tile([128, TOK], BF16, tag="t1")
                e_sub.tensor_sub(out=t1, in0=xin.bitcast(FP32),
                                 in1=mean_b)
                t2 = tmpp.tile([128, TOK], BF16, tag="t2")
                e_mul.tensor_mul(out=t2, in0=t1, in1=rstd_b)
                o = out_pool.tile([128, TOK], BF16, tag=f"{out_tag}{ct}")
                if ct % 2 == 0:
                    nc.scalar.activation(out=o, in_=t2, func=AF.Identity,
                                         scale=g_s[:, ct:ct + 1],
                                         bias=be_s[:, ct:ct + 1])
                else:
                    nc.vector.tensor_scalar(out=o, in0=t2,
                                            scalar1=g_s[:, ct:ct + 1],
                                            scalar2=be_s[:, ct:ct + 1],
                                            op0=ALU.mult, op1=ALU.add)
                outs.append(o)
            return outs

        # right-side persistent pools: o_dr, proj streams, xt (LN1 out), q
        odr_stack = ExitStack()
        o_pool = odr_stack.enter_context(
            tc.tile_pool(name="odr", bufs=1, side="right"))
        pjs_stack = ExitStack()
        wp_pool = pjs_stack.enter_context(
            tc.tile_pool(name="wp", bufs=2, side="right"))
        xr_pool = pjs_stack.enter_context(
            tc.tile_pool(name="xr", bufs=4, side="right"))
        xt_stack = ExitStack()
        xtp = xt_stack.enter_context(
            tc.tile_pool(name="xt", bufs=1, side="right"))
        q_stack = ExitStack()
        q_pool = q_stack.enter_context(
            tc.tile_pool(name="qT", bufs=1, side="right"))

        # pair-tiles (2 c-planes each): finer deps let consumers start as
        # soon as their pair is written, instead of waiting on a whole
        # [128, CT, TOK] tile
        xt_pairs = [xtp.tile([128, 2, TOK], FP8, tag=f"xtdr{p}",
                             name=f"xtdr{p}") for p in range(NT)]
        o_pairs = [o_pool.tile([128, 2, TOK], FP8, tag=f"odr{p}",
                               name=f"odr{p}") for p in range(NT)]

        # ================= LN1 (fp8 x, DR stats) =================
        # LN1's wall time is DVE/ACT elementwise work ([128,1024] ops at
        # ~0.4-1.1us each), so spread squares + normalize across the DVE,
        # GpSimd and ACT engines instead of serializing on two of them.
        with ExitStack() as ph:
            x8p = ph.enter_context(tc.tile_pool(name="x8", bufs=1))
            sqp = ph.enter_context(tc.tile_pool(name="sq8", bufs=1))
            ln_ps = ph.enter_context(
                tc.tile_pool(name="lnps", bufs=4, space="PSUM"))
            vecp = ph.enter_context(tc.tile_pool(name="ve1", bufs=4))
            tmpp = ph.enter_context(tc.tile_pool(name="tm1", bufs=6))

            x8t = [x8p.tile([128, 2, TOK], FP8, tag=f"x8t{p}",
                            name=f"x8t{p}") for p in range(NT)]
            sq8 = [sqp.tile([128, 2, TOK], FP8, tag=f"sq8t{p}",
                            name=f"sq8t{p}") for p in range(NT)]
            for p in range(NT):
                eng = nc.sync if p % 2 == 0 else nc.gpsimd
                eng.dma_start(out=x8t[p], in_=x8d[:, 2 * p:2 * p + 2, :])
            for ct in range(CT):
                xs = x8t[ct // 2][:, ct % 2, :]
                ss = sq8[ct // 2][:, ct % 2, :]
                if ct % 2 == 0:
                    nc.scalar.activation(out=ss, in_=xs, func=AF.Square)
                else:
                    nc.vector.tensor_mul(out=ss, in0=xs, in1=xs)
            ps_sum = [ln_ps.tile([128, NH], FP32, tag="ln",
                                 name=f"l1s{i}") for i in range(2)]
            ps_sq = [ln_ps.tile([128, NH], FP32, tag="ln",
                                name=f"l1q{i}") for i in range(2)]
            for t in range(NT):
                for i in range(2):
                    sl = slice(i * NH, (i + 1) * NH)
                    nc.tensor.matmul(out=ps_sum[i], lhsT=ones8,
                                     rhs=x8t[t][:, :, sl],
                                     start=(t == 0), stop=(t == NT - 1),
                                     perf_mode=DR)
                    nc.tensor.matmul(out=ps_sq[i], lhsT=ones8,
                                     rhs=sq8[t][:, :, sl],
                                     start=(t == 0), stop=(t == NT - 1),
                                     perf_mode=DR)
            mean_b = vecp.tile([128, TOK], FP32, tag="v")
            ex2 = vecp.tile([128, TOK], FP32, tag="v")
            for i in range(2):
                sl = slice(i * NH, (i + 1) * NH)
                nc.vector.tensor_scalar_mul(out=mean_b[:, sl],
                                            in0=ps_sum[i],
                                            scalar1=1.0 / DIM)
                nc.vector.tensor_scalar_mul(out=ex2[:, sl], in0=ps_sq[i],
                                            scalar1=1.0 / DIM)
            var = vecp.tile([128, TOK], FP32, tag="v")
            nc.vector.scalar_tensor_tensor(out=var, in0=mean_b, scalar=-1.0,
                                           in1=mean_b, op0=ALU.mult,
                                           op1=ALU.mult)
            nc.vector.tensor_add(out=var, in0=var, in1=ex2)
            lnv = vecp.tile([128, TOK], FP32, tag="v")
            nc.scalar.activation(out=lnv, in_=var, func=AF.Ln, bias=eps128)
            rstd_b = vecp.tile([128, TOK], FP32, tag="v")
            nc.scalar.activation(out=rstd_b, in_=lnv, func=AF.Exp,
                                 scale=-0.5)
            mean_u = vecp.tile([128, TOK], BF16, tag="vbf")
            nc.gpsimd.tensor_copy(out=mean_u, in_=mean_b)
            rstd_u = vecp.tile([128, TOK], BF16, tag="vbf")
            nc.gpsimd.tensor_copy(out=rstd_u, in_=rstd_b)
            for ct in range(CT):
                xs = x8t[ct // 2][:, ct % 2, :]
                xo = xt_pairs[ct // 2][:, ct % 2, :]
                e_sub = nc.vector if ct % 2 == 0 else nc.gpsimd
                e_mul = nc.gpsimd if ct % 2 == 0 else nc.vector
                t1 = tmpp.tile([128, TOK], BF16, tag="t1")
                e_sub.tensor_sub(out=t1, in0=xs, in1=mean_u)
                t2 = tmpp.tile([128, TOK], BF16, tag="t2")
                e_mul.tensor_mul(out=t2, in0=t1, in1=rstd_u)
                if ct % 2 == 0:
                    nc.scalar.activation(out=xo, in_=t2, func=AF.Identity,
                                         scale=g1s[:, ct:ct + 1],
                                         bias=be1s[:, ct:ct + 1])
                else:
                    nc.vector.tensor_scalar(out=xo, in0=t2,
                                            scalar1=g1s[:, ct:ct + 1],
                                            scalar2=be1s[:, ct:ct + 1],
                                            op0=ALU.mult, op1=ALU.add)

        # ================= QKV (DoubleRow fp8) =================
        kv_stack = ExitStack()
        k_pool = kv_stack.enter_context(tc.tile_pool(name="kp", bufs=1))
        v_pool = kv_stack.enter_context(tc.tile_pool(name="vp", bufs=1))
        k_tiles = {0: {}, 1: {}}
        v_tiles = {0: [], 1: []}
        q_tiles = {}
        for j in range(2):
            v_tiles[j] = [v_pool.tile([128, DIM], BF16, tag=f"v{j}_{tt}",
                                      name=f"vt{j}_{tt}")
                          for tt in range(4)]
        with ExitStack() as qk:
            wt_pool = qk.enter_context(tc.tile_pool(name="wqkv", bufs=3))
            qkv_ps = qk.enter_context(
                tc.tile_pool(name="qkvps", bufs=8, space="PSUM"))

            # k: for each og, 8 accumulators (4 o-tiles x 2 clips)
            for og in range(4):
                wt = wt_pool.tile([128, CT, 512], FP8, tag="w")
                nc.sync.dma_start(
                    out=wt, in_=wk8d[:, og * 8192:(og + 1) * 8192])
                pss = {}
                for ot in range(4):
                    for j in range(2):
                        pss[(ot, j)] = qkv_ps.tile(
                            [128, NH], FP32, tag="qkv",
                            name=f"psk{og}_{ot}_{j}")
                for t in range(NT):
                    for ot in range(4):
                        for j in range(2):
                            nc.tensor.matmul(
                                out=pss[(ot, j)],
                                lhsT=wt[:, 2 * t:2 * t + 2,
                                        ot * 128:(ot + 1) * 128],
                                rhs=xt_pairs[t][:, :,
                                               j * NH:(j + 1) * NH],
                                start=(t == 0), stop=(t == NT - 1),
                                perf_mode=DR)
                for ot in range(4):
                    go = og * 4 + ot
                    for j in range(2):
                        kt = k_pool.tile([128, NH], BF16, tag=f"k{j}_{go}",
                                         name=f"kt{j}_{go}")
                        nc.any.tensor_copy(out=kt, in_=pss[(ot, j)])
                        k_tiles[j][go] = kt

            # q (clip0 only)
            for og in range(4):
                wt = wt_pool.tile([128, CT, 512], FP8, tag="w")
                nc.sync.dma_start(
                    out=wt, in_=wq8d[:, og * 8192:(og + 1) * 8192])
                pss = [qkv_ps.tile([128, NH], FP32, tag="qkv",
                                   name=f"psq{og}_{i}") for i in range(4)]
                for t in range(NT):
                    for ot in range(4):
                        nc.tensor.matmul(
                            out=pss[ot],
                            lhsT=wt[:, 2 * t:2 * t + 2,
                                    ot * 128:(ot + 1) * 128],
                            rhs=xt_pairs[t][:, :, 0:NH],
                            start=(t == 0), stop=(t == NT - 1),
                            perf_mode=DR)
                for ot in range(4):
                    go = og * 4 + ot
                    qt = q_pool.tile([128, NH], BF16, tag=f"q{go}",
                                     name=f"qt{go}")
                    nc.any.tensor_copy(out=qt, in_=pss[ot])
                    q_tiles[go] = qt

            # v token-major: for each vg, 8 accumulators (4 tok-tiles x 2)
            for vg in range(4):
                wt = wt_pool.tile([128, CT, 512], FP8, tag="w")
                nc.gpsimd.dma_start(
                    out=wt, in_=wv8d[:, vg * 8192:(vg + 1) * 8192])
                psv = {}
                for tt in range(4):
                    for j in range(2):
                        psv[(tt, j)] = qkv_ps.tile(
                            [128, 512], FP32, tag="qkv",
                            name=f"psv{vg}_{tt}_{j}")
                for t in range(NT):
                    for tt in range(4):
                        for j in range(2):
                            c0 = j * NH + tt * 128
                            nc.tensor.matmul(
                                out=psv[(tt, j)],
                                lhsT=xt_pairs[t][:, :, c0:c0 + 128],
                                rhs=wt[:, 2 * t:2 * t + 2, :],
                                start=(t == 0), stop=(t == NT - 1),
                                perf_mode=DR)
                for tt in range(4):
                    for j in range(2):
                        nc.any.tensor_copy(
                            out=v_tiles[j][tt][:, vg * 512:(vg + 1) * 512],
                            in_=psv[(tt, j)])

        # ================= Attention (clips interleaved) =================
        with ExitStack() as at:
            e_pool = at.enter_context(tc.tile_pool(name="ex", bufs=8))
            bcp = at.enter_context(tc.tile_pool(name="ab", bufs=3))
            s_ps = at.enter_context(
                tc.tile_pool(name="sps", bufs=4, space="PSUM"))
            sum_ps = at.enter_context(
                tc.tile_pool(name="sums", bufs=2, space="PSUM"))
            o_ps = at.enter_context(
                tc.tile_pool(name="ops", bufs=2, space="PSUM"))
            for h in range(HEADS):
                qh = q_tiles[h]
                for j in range(2):
                    c0 = j * NH
                    exps = []
                    for mt in range(4):
                        ps_s = s_ps.tile([128, NH], FP32, tag="s")
                        nc.tensor.matmul(
                            out=ps_s,
                            lhsT=k_tiles[j][h][:, mt * 128:(mt + 1) * 128],
                            rhs=qh, start=True, stop=True)
                        e = e_pool.tile([128, NH], BF16, tag="e")
                        nc.scalar.activation(out=e, in_=ps_s, func=AF.Exp,
                                             scale=SCALE / (WS * WS))
                        exps.append(e)
                    ps_sum = sum_ps.tile([128, NH], FP32, tag="as")
                    for mt in range(4):
                        nc.tensor.matmul(out=ps_sum, lhsT=onesm_bf,
                                         rhs=exps[mt],
                                         start=(mt == 0), stop=(mt == 3))
                    r_b = bcp.tile([128, NH], FP32, tag="rb")
                    nc.vector.reciprocal_approx_fast(out=r_b, in_=ps_sum)
                    ps_o = o_ps.tile([128, NH], FP32, tag="o")
                    for mt in range(4):
                        nc.tensor.matmul(
                            out=ps_o,
                            lhsT=v_tiles[j][mt][:, h * 128:(h + 1) * 128],
                            rhs=exps[mt], start=(mt == 0), stop=(mt == 3))
                    nc.vector.tensor_mul(
                        out=o_pairs[h // 2][:, h % 2, c0:c0 + NH],
                        in0=ps_o, in1=r_b)
        kv_stack.close()
        q_stack.close()
        xt_stack.close()

        # ================= Projection + residual =================
        # psum carries WS*VS = 512 (o x16, wp x32); bproj folded into xT
        xmid_stack = ExitStack()
        xm_pool = xmid_stack.enter_context(tc.tile_pool(name="xmid", bufs=1))
        xt2_stack = ExitStack()
        xt2_pool = xt2_stack.enter_context(tc.tile_pool(name="xt2", bufs=1))
        ln2_stack = ExitStack()
        ln2_sq = ln2_stack.enter_context(tc.tile_pool(name="sql2", bufs=3))
        ln2_tm = ln2_stack.enter_context(tc.tile_pool(name="tml2", bufs=3))
        ln2_ve = ln2_stack.enter_context(tc.tile_pool(name="vel2", bufs=4))
        xm = [xm_pool.tile([128, TOK], FP32R, tag=f"xm{ct}", name=f"xm{ct}")
              for ct in range(CT)]
        with ExitStack() as ph:
            pj_ps = ln2_stack.enter_context(
                tc.tile_pool(name="pjps", bufs=8, space="PSUM"))
            for og in range(4):
                wt = wp_pool.tile([128, CT, 512], FP8, tag="wp")
                nc.sync.dma_start(
                    out=wt, in_=wp8d[:, og * 8192:(og + 1) * 8192])
                pss = {}
                for nh in range(2):
                    for ot in range(4):
                        pss[(nh, ot)] = pj_ps.tile(
                            [128, NH], FP32, tag="pj",
                            name=f"pspj{og}_{nh}_{ot}")
                for t in range(NT):
                    for nh in range(2):
                        c0 = nh * NH
                        for ot in range(4):
                            nc.tensor.matmul(
                                out=pss[(nh, ot)],
                                lhsT=wt[:, 2 * t:2 * t + 2,
                                        ot * 128:(ot + 1) * 128],
                                rhs=o_pairs[t][:, :, c0:c0 + NH],
                                start=(t == 0), stop=(t == NT - 1),
                                perf_mode=DR)
                for nh in range(2):
                    c0 = nh * NH
                    for ot in range(4):
                        go = og * 4 + ot
                        xr = xr_pool.tile([128, NH], FP32, tag="xr")
                        nc.gpsimd.dma_start(
                            out=xr,
                            in_=xT[go * 128:(go + 1) * 128, c0:c0 + NH])
                        nc.vector.scalar_tensor_tensor(
                            out=xm[go][:, c0:c0 + NH],
                            in0=pss[(nh, ot)],
                            scalar=1.0 / (WS * VS),
                            in1=xr, op0=ALU.mult, op1=ALU.add)

        pjs_stack.close()
        odr_stack.close()

        # ============ LN2 (+ fold b2 into x_mid in place) ============
        def m_loader(ct):
            return xm[ct]

        xt2w = layernorm_wide("l2", m_loader, g2s, be2s, xt2_pool, "x2",
                              pools=(pj_ps, ln2_sq, ln2_tm, ln2_ve))
        ln2_stack.close()
        xt2 = {0: [t[:, 0:NH] for t in xt2w], 1: [t[:, NH:TOK] for t in xt2w]}
        for ct in range(CT):
            nc.vector.tensor_scalar_add(
                out=xm[ct],
                in0=xm[ct].bitcast(FP32),
                scalar1=b2s[:, ct:ct + 1])

        # ================= MLP =================
        with ExitStack() as ph:
            w1_pool = ph.enter_context(tc.tile_pool(name="w1s", bufs=6))
            w2_pool = ph.enter_context(tc.tile_pool(name="w2s", bufs=CH + 1))
            h1_pool = ph.enter_context(
                tc.tile_pool(name="h1", bufs=2 * CH + 2))
            mlp_ps = ph.enter_context(
                tc.tile_pool(name="mlpps", bufs=8, space="PSUM"))
            for fc in range(FT // CH):
                h1 = {}
                for half in range(CH // 4):
                    f0 = fc * CH + half * 4
                    psh = {}
                    for fi in range(4):
                        for nh in range(2):
                            psh[(fi, nh)] = mlp_ps.tile(
                                [128, NH], FP32, tag="mlp",
                                name=f"psh{fc}_{half}_{fi}_{nh}")
                    for ct in range(CT):
                        wt = w1_pool.tile([128, 512], BF16, tag="w1")
                        nc.sync.dma_start(
                            out=wt,
                            in_=w1T[ct * 128:(ct + 1) * 128,
                                    f0 * 128:(f0 + 4) * 128])
                        for fi in range(4):
                            for nh in range(2):
                                nc.tensor.matmul(
                                    out=psh[(fi, nh)],
                                    lhsT=wt[:, fi * 128:(fi + 1) * 128],
                                    rhs=xt2[nh][ct],
                                    start=(ct == 0), stop=(ct == CT - 1))
                    for fi in range(4):
                        f = f0 + fi
                        for nh in range(2):
                            ht = h1_pool.tile([128, NH], BF16, tag="h1")
                            nc.scalar.activation(out=ht, in_=psh[(fi, nh)],
                                                 func=AF.Gelu,
                                                 bias=b1s[:, f:f + 1])
                            h1[(nh, half * 4 + fi)] = ht
                for qd in range(4):
                    w2ts = []
                    for fi in range(CH):
                        f = fc * CH + fi
                        wt = w2_pool.tile([128, 512], BF16, tag="w2")
                        nc.gpsimd.dma_start(
                            out=wt,
                            in_=w2T[f * 128:(f + 1) * 128,
                                    qd * 512:(qd + 1) * 512])
                        w2ts.append(wt)
                    for nh in range(2):
                        c0 = nh * NH
                        pss = [mlp_ps.tile([128, NH], FP32, tag="mlp",
                                           name=f"psw2_{fc}_{qd}_{nh}_{i}")
                               for i in range(4)]
                        for fi in range(CH):
                            for ot in range(4):
                                nc.tensor.matmul(
                                    out=pss[ot],
                                    lhsT=w2ts[fi][:, ot * 128:(ot + 1) * 128],
                                    rhs=h1[(nh, fi)],
                                    start=(fi == 0), stop=(fi == CH - 1))
                        for ot in range(4):
                            go = qd * 4 + ot
                            nc.vector.tensor_add(
                                out=xm[go][:, c0:c0 + NH],
                                in0=xm[go][:, c0:c0 + NH].bitcast(FP32),
                                in1=pss[ot])
        xt2_stack.close()

        # ================= Output =================
        # all on sync: gpsimd still drains the w2 stream at kernel end,
        # and in-order queues would stall the output behind it
        for ct in range(CT):
            nc.sync.dma_start(
                out=outT[ct * 128:(ct + 1) * 128, :],
                in_=xm[ct].bitcast(FP32))
        xmid_stack.close()

    _orig_tables = _patch_act_tables()
    try:
        nc.compile()
    finally:
        import concourse.bacc as _bacc
        _bacc.get_activation_tables = _orig_tables
    return nc


_NC = None


def _get_nc():
    global _NC
    if _NC is None:
        _NC = build()
    return _NC


def _to_fp8(a, scale=1.0):
    import ml_dtypes
    a = np.asarray(a, dtype=np.float32) * scale
    return np.ascontiguousarray(
        np.clip(a, -240.0, 240.0).astype(ml_dtypes.float8_e4m3))


def _dr_planes(wT, scale):
    """[DIM, 2048-out] weight (contraction-major) -> fp8 [128, 4*CT*512]
    laid out [c_inner(128), og(4), ct(16), oc(512)]."""
    w = np.asarray(wT, dtype=np.float32)
    w = w.reshape(CT, 128, 4, 512).transpose(1, 2, 0, 3).reshape(128, -1)
    return _to_fp8(w, scale)


def _prep_shared(Wqkv, Wproj, gamma1, beta1, gamma2, beta2, W1, b1, W2, b2):
    import ml_dtypes

    def f32(a):
        return np.ascontiguousarray(np.asarray(a, dtype=np.float32))

    def bf16(a):
        return np.ascontiguousarray(
            np.asarray(a, dtype=np.float32).astype(ml_dtypes.bfloat16))

    Wqkv = np.asarray(Wqkv)
    return {
        "wq8": _dr_planes(Wqkv[0:DIM].T, WS),
        "wk8": _dr_planes(Wqkv[DIM:2 * DIM].T, WS),
        "wv8": _dr_planes(Wqkv[2 * DIM:3 * DIM].T, VS),
        "wp8": _dr_planes(np.asarray(Wproj).T, WS),
        "w1T": bf16(np.asarray(W1).T),
        "w2T": bf16(np.asarray(W2).T),
        "g1v": f32(np.asarray(gamma1).reshape(CT, 128).T),
        "be1v": f32(np.asarray(beta1).reshape(CT, 128).T),
        "g2v": f32(np.asarray(gamma2).reshape(CT, 128).T),
        "be2v": f32(np.asarray(beta2).reshape(CT, 128).T),
        "b1v": f32(np.asarray(b1).reshape(FT, 128).T),
        "b2v": f32(np.asarray(b2).reshape(CT, 128).T),
    }


def build_in_maps(x, gamma1, beta1, Wqkv, Wproj, bproj, gamma2, beta2, W1,
                  b1, W2, b2):
    x = np.asarray(x, dtype=np.float32)          # [8, 2, 512, 2048]
    bproj = np.asarray(bproj, dtype=np.float32)
    shared = _prep_shared(Wqkv, Wproj, gamma1, beta1, gamma2, beta2,
                          W1, b1, W2, b2)
    in_maps = []
    for i in range(N_CORES):
        xt = np.ascontiguousarray(x[i].reshape(TOK, DIM).T)
        x8 = np.ascontiguousarray(
            xt.reshape(CT, 128, TOK).transpose(1, 0, 2))
        m = {"xT": np.ascontiguousarray(xt + bproj[:, None]),
             "x8": _to_fp8(x8)}
        m.update(shared)
        in_maps.append(m)
    return in_maps


def kernel(x, gamma1, beta1, Wqkv, Wproj, bproj, gamma2, beta2, W1, b1, W2,
           b2):
    nc = _get_nc()
    in_maps = build_in_maps(x, gamma1, beta1, Wqkv, Wproj, bproj, gamma2,
                            beta2, W1, b1, W2, b2)
    res = run_bass_kernel_spmd(nc, in_maps, core_ids=list(range(N_CORES)))
    out = np.stack([
        np.ascontiguousarray(res.results[i]["out"].T).reshape(2, NH, DIM)
        for i in range(N_CORES)
    ])
    return out


# revision 17
# speedup vs baseline: 1.0315x; 1.0315x over previous
"""Trainium2 Bass kernel for nn_Block_420906795461 (dense transformer block).

Data-parallel over B=8 clip-pairs across 8 NeuronCores; each core runs the
full block on its [2, 512, 2048] shard with replicated weights; no
collectives.

v2: fp8e4 DoubleRow (2 MACs/PE/cycle) on the precision-safe matmuls —
LN1 stats, QKV (q, k, v) and the attention output projection — with
host-side fp8 weight planes and fp8 activations (LN1 output, attention
output). Attention scores / softmax / AV and the whole MLP stay bf16:
numpy simulation showed e4m3 noise there breaks the 2e-2 gate (exp
amplifies score errors; quantized exp weights flip near-tie attention
mixes; MLP output noise 2.5%+ of a branch that carries half the output
variance). Measured config error (sim): rel ~9e-3 vs gate 2e-2.

Scale carrying: wq/wk/wp prescaled x32, wv x16 (host). q,k bf16 carry
x32 -> exp scale = SCALE/1024. v carries x16 -> attention out fp8
carries x16 -> proj psum carries x512, divided out at the residual add.
bproj is folded into the residual input xT host-side.

Per-core dataflow (activations feature-major [feature, token] in SBUF,
tokens 0..511 = clip0, 512..1023 = clip1):
  LN1 (fp8 x, DR stats) -> QKV DR-fp8 -> attention bf16 (per-head,
  clips interleaved; out -> fp8 o_dr in SBUF) -> proj DR-fp8 + residual
  -> LN2 (fp32r stats) -> MLP bf16 (W1 + erf-gelu + W2, PSUM chunks
  added in place into the fp32 residual tiles).

LayerNorm stats and softmax denominators use all-ones stationary
matmuls (partition reduction pre-broadcast). Softmax skips
max-subtraction (scores ~N(0,1)); exp scale and the denominator divide
fold into PSUM-evacuation ops. rstd via exp(-0.5*ln(var+eps)) keeps one
ACT table pre-MLP.
"""

import sys

import numpy as np

sys.path.insert(0, "/opt/trn_rl_repo")

from contextlib import ExitStack

import concourse.bass as bass  # noqa: F401
import concourse.mybir as mybir
import concourse.tile as tile
from concourse import bacc
from concourse.bass_utils import run_bass_kernel_spmd

FP32 = mybir.dt.float32
FP32R = mybir.dt.float32r
BF16 = mybir.dt.bfloat16
FP8 = mybir.dt.float8e4
AF = mybir.ActivationFunctionType
ALU = mybir.AluOpType
DR = mybir.MatmulPerfMode.DoubleRow

DIM = 2048
HEADS = 16
HD = 128
F = 4 * DIM          # 8192
TOK = 1024           # tokens per core (2 clips x 512)
NH = 512             # tokens per clip
CT = DIM // 128      # 16 c-tiles
NT = CT // 2         # 8 DoubleRow k-steps over c
FT = F // 128        # 64 f-tiles
CH = 16              # mlp chunk size in f-tiles
SCALE = HD ** -0.5
EPS = 1e-5
N_CORES = 8
WS = 32.0            # fp8 prescale for wq/wk/wp
VS = 16.0            # fp8 prescale for wv (v + attn out carry x16)


def _patch_act_tables():
    """Force the ACT-table chooser onto two sets that jointly cover every
    activation function this kernel uses ({Square, Ln, Exp, Identity, Copy}
    pre-MLP, + Gelu), so the table is swapped once instead of ~6 times
    (each mid-kernel ACT_TABLE_LOAD is a 1.28us stall in a serial chain).
    Set ids are positional, so unwanted sets are blanked rather than
    removed."""
    import concourse.bacc as _bacc

    _orig = _bacc.get_activation_tables
    keep = {"natural_log_exp_and_others", "gelu_and_others"}

    def patched(arch):
        return {name: (funcs if name in keep else set())
                for name, funcs in _orig(arch).items()}

    _bacc.get_activation_tables = patched
    return _orig


def build():
    nc = bacc.Bacc("TRN2", target_bir_lowering=False, debug=False)

    xT = nc.dram_tensor("xT", [DIM, TOK], FP32, kind="ExternalInput").ap()
    x8d = nc.dram_tensor("x8", [128, CT, TOK], FP8,
                         kind="ExternalInput").ap()
    wq8d = nc.dram_tensor("wq8", [128, 4 * CT * 512], FP8,
                          kind="ExternalInput").ap()
    wk8d = nc.dram_tensor("wk8", [128, 4 * CT * 512], FP8,
                          kind="ExternalInput").ap()
    wv8d = nc.dram_tensor("wv8", [128, 4 * CT * 512], FP8,
                          kind="ExternalInput").ap()
    wp8d = nc.dram_tensor("wp8", [128, 4 * CT * 512], FP8,
                          kind="ExternalInput").ap()
    w1T = nc.dram_tensor("w1T", [DIM, F], BF16, kind="ExternalInput").ap()
    w2T = nc.dram_tensor("w2T", [F, DIM], BF16, kind="ExternalInput").ap()
    g1v = nc.dram_tensor("g1v", [128, CT], FP32, kind="ExternalInput").ap()
    be1v = nc.dram_tensor("be1v", [128, CT], FP32, kind="ExternalInput").ap()
    g2v = nc.dram_tensor("g2v", [128, CT], FP32, kind="ExternalInput").ap()
    be2v = nc.dram_tensor("be2v", [128, CT], FP32, kind="ExternalInput").ap()
    b1v = nc.dram_tensor("b1v", [128, FT], FP32, kind="ExternalInput").ap()
    b2v = nc.dram_tensor("b2v", [128, CT], FP32, kind="ExternalInput").ap()
    outT = nc.dram_tensor("out", [DIM, TOK], FP32, kind="ExternalOutput").ap()

    with tile.TileContext(nc, pool_alloc_mode="stack") as tc, \
            ExitStack() as top:
        consts = top.enter_context(tc.tile_pool(name="consts", bufs=1))

        onesm_bf = consts.tile([128, 128], BF16, tag="onesmbf")
        nc.vector.memset(onesm_bf, 1.0)
        onesm_f = consts.tile([128, 128], FP32, tag="onesmf")
        nc.vector.memset(onesm_f, 1.0)
        onesm_r = onesm_f.bitcast(FP32R)
        ones8 = consts.tile([128, 2, 128], FP8, tag="ones8")
        nc.vector.memset(ones8, 1.0)
        eps128 = consts.tile([128, 1], FP32, tag="eps")
        nc.vector.memset(eps128, EPS)
        # dummy ACT op: triggers the (single) pre-MLP ACT table load during
        # the initial input DMAs instead of stalling LN1's first Square
        warm = consts.tile([128, 1], FP32, tag="warm")
        nc.scalar.activation(out=warm, in_=eps128, func=AF.Exp)

        def load_const(name, src, cols):
            t = consts.tile([128, cols], FP32, tag=name)
            nc.sync.dma_start(out=t, in_=src)
            return t

        g1s = load_const("g1s", g1v, CT)
        be1s = load_const("be1s", be1v, CT)
        g2s = load_const("g2s", g2v, CT)
        be2s = load_const("be2s", be2v, CT)
        b1s = load_const("b1s", b1v, FT)
        b2s = load_const("b2s", b2v, CT)

        def layernorm_wide(uid, loader, g_s, be_s, out_pool, out_tag,
                           pools):
            """Full-width (TOK) LN over fp32r input tiles (LN2).
            loader(ct) returns a [128, TOK] fp32r AP. Returns 16 bf16
            tiles. pools=(ln_ps, sqp, tmpp, vecp) reuses pre-opened pools
            so the stats can overlap a preceding phase."""
            ln_ps, sqp, tmpp, vecp = pools
            ps_sum = [ln_ps.tile([128, NH], FP32, tag="pj",
                                 name=f"lns{uid}{i}") for i in range(2)]
            ps_sq = [ln_ps.tile([128, NH], FP32, tag="pj",
                                name=f"lnq{uid}{i}") for i in range(2)]
            for ct in range(CT):
                xin = loader(ct)
                sq = sqp.tile([128, TOK], FP32R, tag="sq")
                nc.scalar.activation(out=sq, in_=xin.bitcast(FP32),
                                     func=AF.Square)
                for i in range(2):
                    sl = slice(i * NH, (i + 1) * NH)
                    nc.tensor.matmul(out=ps_sum[i], lhsT=onesm_r,
                                     rhs=xin[:, sl],
                                     start=(ct == 0), stop=(ct == CT - 1))
                    nc.tensor.matmul(out=ps_sq[i], lhsT=onesm_r,
                                     rhs=sq[:, sl],
                                     start=(ct == 0), stop=(ct == CT - 1))
            mean_b = vecp.tile([128, TOK], FP32, tag="v")
            ex2 = vecp.tile([128, TOK], FP32, tag="v")
            for i in range(2):
                sl = slice(i * NH, (i + 1) * NH)
                nc.vector.tensor_scalar_mul(out=mean_b[:, sl],
                                            in0=ps_sum[i],
                                            scalar1=1.0 / DIM)
                nc.vector.tensor_scalar_mul(out=ex2[:, sl],
                                            in0=ps_sq[i],
                                            scalar1=1.0 / DIM)
            msq = vecp.tile([128, TOK], FP32, tag="v")
            nc.vector.tensor_mul(out=msq, in0=mean_b, in1=mean_b)
            var = vecp.tile([128, TOK], FP32, tag="v")
            nc.vector.tensor_sub(out=var, in0=ex2, in1=msq)
            # rstd = (var+eps)^-0.5 = exp(-0.5*ln(var+eps)); keeps the
            # whole pre-MLP kernel on one ACT table (no Sqrt swap-ins)
            lnv = vecp.tile([128, TOK], FP32, tag="v")
            nc.scalar.activation(out=lnv, in_=var, func=AF.Ln,
                                 bias=eps128)
            rstd_b = vecp.tile([128, TOK], FP32, tag="v")
            nc.scalar.activation(out=rstd_b, in_=lnv, func=AF.Exp,
                                 scale=-0.5)
            outs = []
            for ct in range(CT):
                xin = loader(ct)
                t1 = tmpp.tile([128, TOK], BF16, tag="t1")
                nc.vector.tensor_sub(out=t1, in0=xin.bitcast(FP32),
                                     in1=mean_b)
                t2 = tmpp.tile([128, TOK], BF16, tag="t2")
                nc.vector.tensor_mul(out=t2, in0=t1, in1=rstd_b)
                o = out_pool.tile([128, TOK], BF16, tag=f"{out_tag}{ct}")
                nc.scalar.activation(out=o, in_=t2, func=AF.Identity,
                                     scale=g_s[:, ct:ct + 1],
                                     bias=be_s[:, ct:ct + 1])
                outs.append(o)
            return outs

        # right-side persistent pools: o_dr, proj streams, xt (LN1 out), q
        odr_stack = ExitStack()
        o_pool = odr_stack.enter_context(
            tc.tile_pool(name="odr", bufs=1, side="right"))
        pjs_stack = ExitStack()
        wp_pool = pjs_stack.enter_context(
            tc.tile_pool(name="wp", bufs=2, side="right"))
        xr_pool = pjs_stack.enter_context(
            tc.tile_pool(name="xr", bufs=4, side="right"))
        xt_stack = ExitStack()
        xtp = xt_stack.enter_context(
            tc.tile_pool(name="xt", bufs=1, side="right"))
        q_stack = ExitStack()
        q_pool = q_stack.enter_context(
            tc.tile_pool(name="qT", bufs=1, side="right"))

        xt_dr = xtp.tile([128, CT, TOK], FP8, tag="xtdr")
        o_dr = o_pool.tile([128, CT, TOK], FP8, tag="odr")

        # ================= LN1 (fp8 x, DR stats) =================
        with ExitStack() as ph:
            x8p = ph.enter_context(tc.tile_pool(name="x8", bufs=1))
            sqp = ph.enter_context(tc.tile_pool(name="sq8", bufs=1))
            ln_ps = ph.enter_context(
                tc.tile_pool(name="lnps", bufs=4, space="PSUM"))
            vecp = ph.enter_context(tc.tile_pool(name="ve1", bufs=4))
            tmpp = ph.enter_context(tc.tile_pool(name="tm1", bufs=3))

            x8t = x8p.tile([128, CT, TOK], FP8, tag="x8t")
            for c4 in range(4):
                eng = nc.sync if c4 % 2 == 0 else nc.gpsimd
                eng.dma_start(out=x8t[:, c4 * 4:(c4 + 1) * 4, :],
                              in_=x8d[:, c4 * 4:(c4 + 1) * 4, :])
            sq8 = sqp.tile([128, CT, TOK], FP8, tag="sq8t")
            for ct in range(CT):
                nc.scalar.activation(out=sq8[:, ct, :], in_=x8t[:, ct, :],
                                     func=AF.Square)
            ps_sum = [ln_ps.tile([128, NH], FP32, tag="ln",
                                 name=f"l1s{i}") for i in range(2)]
            ps_sq = [ln_ps.tile([128, NH], FP32, tag="ln",
                                name=f"l1q{i}") for i in range(2)]
            for t in range(NT):
                for i in range(2):
                    sl = slice(i * NH, (i + 1) * NH)
                    nc.tensor.matmul(out=ps_sum[i], lhsT=ones8,
                                     rhs=x8t[:, 2 * t:2 * t + 2, sl],
                                     start=(t == 0), stop=(t == NT - 1),
                                     perf_mode=DR)
                    nc.tensor.matmul(out=ps_sq[i], lhsT=ones8,
                                     rhs=sq8[:, 2 * t:2 * t + 2, sl],
                                     start=(t == 0), stop=(t == NT - 1),
                                     perf_mode=DR)
            mean_b = vecp.tile([128, TOK], FP32, tag="v")
            ex2 = vecp.tile([128, TOK], FP32, tag="v")
            for i in range(2):
                sl = slice(i * NH, (i + 1) * NH)
                nc.vector.tensor_scalar_mul(out=mean_b[:, sl],
                                            in0=ps_sum[i],
                                            scalar1=1.0 / DIM)
                nc.vector.tensor_scalar_mul(out=ex2[:, sl], in0=ps_sq[i],
                                            scalar1=1.0 / DIM)
            msq = vecp.tile([128, TOK], FP32, tag="v")
            nc.vector.tensor_mul(out=msq, in0=mean_b, in1=mean_b)
            var = vecp.tile([128, TOK], FP32, tag="v")
            nc.vector.tensor_sub(out=var, in0=ex2, in1=msq)
            lnv = vecp.tile([128, TOK], FP32, tag="v")
            nc.scalar.activation(out=lnv, in_=var, func=AF.Ln, bias=eps128)
            rstd_b = vecp.tile([128, TOK], FP32, tag="v")
            nc.scalar.activation(out=rstd_b, in_=lnv, func=AF.Exp,
                                 scale=-0.5)
            mean_u = vecp.tile([128, TOK], BF16, tag="vbf")
            nc.scalar.copy(out=mean_u, in_=mean_b)
            rstd_u = vecp.tile([128, TOK], BF16, tag="vbf")
            nc.scalar.copy(out=rstd_u, in_=rstd_b)
            for ct in range(CT):
                t1 = tmpp.tile([128, TOK], BF16, tag="t1")
                nc.vector.tensor_sub(out=t1, in0=x8t[:, ct, :], in1=mean_u)
                t2 = tmpp.tile([128, TOK], BF16, tag="t2")
                nc.vector.tensor_mul(out=t2, in0=t1, in1=rstd_u)
                nc.scalar.activation(out=xt_dr[:, ct, :], in_=t2,
                                     func=AF.Identity,
                                     scale=g1s[:, ct:ct + 1],
                                     bias=be1s[:, ct:ct + 1])

        # ================= QKV (DoubleRow fp8) =================
        kv_stack = ExitStack()
        k_pool = kv_stack.enter_context(tc.tile_pool(name="kp", bufs=1))
        v_pool = kv_stack.enter_context(tc.tile_pool(name="vp", bufs=1))
        k_tiles = {0: {}, 1: {}}
        v_tiles = {0: [], 1: []}
        q_tiles = {}
        for j in range(2):
            v_tiles[j] = [v_pool.tile([128, DIM], BF16, tag=f"v{j}_{tt}",
                                      name=f"vt{j}_{tt}")
                          for tt in range(4)]
        with ExitStack() as qk:
            wt_pool = qk.enter_context(tc.tile_pool(name="wqkv", bufs=3))
            qkv_ps = qk.enter_context(
                tc.tile_pool(name="qkvps", bufs=8, space="PSUM"))

            # k: for each og, 8 accumulators (4 o-tiles x 2 clips)
            for og in range(4):
                wt = wt_pool.tile([128, CT, 512], FP8, tag="w")
                nc.sync.dma_start(
                    out=wt, in_=wk8d[:, og * 8192:(og + 1) * 8192])
                pss = {}
                for ot in range(4):
                    for j in range(2):
                        pss[(ot, j)] = qkv_ps.tile(
                            [128, NH], FP32, tag="qkv",
                            name=f"psk{og}_{ot}_{j}")
                for t in range(NT):
                    for ot in range(4):
                        for j in range(2):
                            nc.tensor.matmul(
                                out=pss[(ot, j)],
                                lhsT=wt[:, 2 * t:2 * t + 2,
                                        ot * 128:(ot + 1) * 128],
                                rhs=xt_dr[:, 2 * t:2 * t + 2,
                                          j * NH:(j + 1) * NH],
                                start=(t == 0), stop=(t == NT - 1),
                                perf_mode=DR)
                for ot in range(4):
                    go = og * 4 + ot
                    for j in range(2):
                        kt = k_pool.tile([128, NH], BF16, tag=f"k{j}_{go}",
                                         name=f"kt{j}_{go}")
                        nc.any.tensor_copy(out=kt, in_=pss[(ot, j)])
                        k_tiles[j][go] = kt

            # q (clip0 only)
            for og in range(4):
                wt = wt_pool.tile([128, CT, 512], FP8, tag="w")
                nc.sync.dma_start(
                    out=wt, in_=wq8d[:, og * 8192:(og + 1) * 8192])
                pss = [qkv_ps.tile([128, NH], FP32, tag="qkv",
                                   name=f"psq{og}_{i}") for i in range(4)]
                for t in range(NT):
                    for ot in range(4):
                        nc.tensor.matmul(
                            out=pss[ot],
                            lhsT=wt[:, 2 * t:2 * t + 2,
                                    ot * 128:(ot + 1) * 128],
                            rhs=xt_dr[:, 2 * t:2 * t + 2, 0:NH],
                            start=(t == 0), stop=(t == NT - 1),
                            perf_mode=DR)
                for ot in range(4):
                    go = og * 4 + ot
                    qt = q_pool.tile([128, NH], BF16, tag=f"q{go}",
                                     name=f"qt{go}")
                    nc.any.tensor_copy(out=qt, in_=pss[ot])
                    q_tiles[go] = qt

            # v token-major: for each vg, 8 accumulators (4 tok-tiles x 2)
            for vg in range(4):
                wt = wt_pool.tile([128, CT, 512], FP8, tag="w")
                nc.gpsimd.dma_start(
                    out=wt, in_=wv8d[:, vg * 8192:(vg + 1) * 8192])
                psv = {}
                for tt in range(4):
                    for j in range(2):
                        psv[(tt, j)] = qkv_ps.tile(
                            [128, 512], FP32, tag="qkv",
                            name=f"psv{vg}_{tt}_{j}")
                for t in range(NT):
                    for tt in range(4):
                        for j in range(2):
                            c0 = j * NH + tt * 128
                            nc.tensor.matmul(
                                out=psv[(tt, j)],
                                lhsT=xt_dr[:, 2 * t:2 * t + 2,
                                           c0:c0 + 128],
                                rhs=wt[:, 2 * t:2 * t + 2, :],
                                start=(t == 0), stop=(t == NT - 1),
                                perf_mode=DR)
                for tt in range(4):
                    for j in range(2):
                        nc.any.tensor_copy(
                            out=v_tiles[j][tt][:, vg * 512:(vg + 1) * 512],
                            in_=psv[(tt, j)])

        # ================= Attention (clips interleaved) =================
        with ExitStack() as at:
            e_pool = at.enter_context(tc.tile_pool(name="ex", bufs=8))
            bcp = at.enter_context(tc.tile_pool(name="ab", bufs=3))
            s_ps = at.enter_context(
                tc.tile_pool(name="sps", bufs=4, space="PSUM"))
            sum_ps = at.enter_context(
                tc.tile_pool(name="sums", bufs=2, space="PSUM"))
            o_ps = at.enter_context(
                tc.tile_pool(name="ops", bufs=2, space="PSUM"))
            for h in range(HEADS):
                qh = q_tiles[h]
                for j in range(2):
                    c0 = j * NH
                    exps = []
                    for mt in range(4):
                        ps_s = s_ps.tile([128, NH], FP32, tag="s")
                        nc.tensor.matmul(
                            out=ps_s,
                            lhsT=k_tiles[j][h][:, mt * 128:(mt + 1) * 128],
                            rhs=qh, start=True, stop=True)
                        e = e_pool.tile([128, NH], BF16, tag="e")
                        nc.scalar.activation(out=e, in_=ps_s, func=AF.Exp,
                                             scale=SCALE / (WS * WS))
                        exps.append(e)
                    ps_sum = sum_ps.tile([128, NH], FP32, tag="as")
                    for mt in range(4):
                        nc.tensor.matmul(out=ps_sum, lhsT=onesm_bf,
                                         rhs=exps[mt],
                                         start=(mt == 0), stop=(mt == 3))
                    r_b = bcp.tile([128, NH], FP32, tag="rb")
                    nc.vector.reciprocal_approx_fast(out=r_b, in_=ps_sum)
                    ps_o = o_ps.tile([128, NH], FP32, tag="o")
                    for mt in range(4):
                        nc.tensor.matmul(
                            out=ps_o,
                            lhsT=v_tiles[j][mt][:, h * 128:(h + 1) * 128],
                            rhs=exps[mt], start=(mt == 0), stop=(mt == 3))
                    nc.vector.tensor_mul(out=o_dr[:, h, c0:c0 + NH],
                                         in0=ps_o, in1=r_b)
        kv_stack.close()
        q_stack.close()
        xt_stack.close()

        # ================= Projection + residual =================
        # psum carries WS*VS = 512 (o x16, wp x32); bproj folded into xT
        xmid_stack = ExitStack()
        xm_pool = xmid_stack.enter_context(tc.tile_pool(name="xmid", bufs=1))
        xt2_stack = ExitStack()
        xt2_pool = xt2_stack.enter_context(tc.tile_pool(name="xt2", bufs=1))
        ln2_stack = ExitStack()
        ln2_sq = ln2_stack.enter_context(tc.tile_pool(name="sql2", bufs=3))
        ln2_tm = ln2_stack.enter_context(tc.tile_pool(name="tml2", bufs=3))
        ln2_ve = ln2_stack.enter_context(tc.tile_pool(name="vel2", bufs=4))
        xm = [xm_pool.tile([128, TOK], FP32R, tag=f"xm{ct}", name=f"xm{ct}")
              for ct in range(CT)]
        with ExitStack() as ph:
            pj_ps = ln2_stack.enter_context(
                tc.tile_pool(name="pjps", bufs=8, space="PSUM"))
            for og in range(4):
                wt = wp_pool.tile([128, CT, 512], FP8, tag="wp")
                nc.sync.dma_start(
                    out=wt, in_=wp8d[:, og * 8192:(og + 1) * 8192])
                pss = {}
                for nh in range(2):
                    for ot in range(4):
                        pss[(nh, ot)] = pj_ps.tile(
                            [128, NH], FP32, tag="pj",
                            name=f"pspj{og}_{nh}_{ot}")
                for t in range(NT):
                    for nh in range(2):
                        c0 = nh * NH
                        for ot in range(4):
                            nc.tensor.matmul(
                                out=pss[(nh, ot)],
                                lhsT=wt[:, 2 * t:2 * t + 2,
                                        ot * 128:(ot + 1) * 128],
                                rhs=o_dr[:, 2 * t:2 * t + 2, c0:c0 + NH],
                                start=(t == 0), stop=(t == NT - 1),
                                perf_mode=DR)
                for nh in range(2):
                    c0 = nh * NH
                    for ot in range(4):
                        go = og * 4 + ot
                        xr = xr_pool.tile([128, NH], FP32, tag="xr")
                        eng = nc.sync if ot % 2 == 0 else nc.gpsimd
                        eng.dma_start(
                            out=xr,
                            in_=xT[go * 128:(go + 1) * 128, c0:c0 + NH])
                        nc.vector.scalar_tensor_tensor(
                            out=xm[go][:, c0:c0 + NH],
                            in0=pss[(nh, ot)],
                            scalar=1.0 / (WS * VS),
                            in1=xr, op0=ALU.mult, op1=ALU.add)

        pjs_stack.close()
        odr_stack.close()

        # ============ LN2 (+ fold b2 into x_mid in place) ============
        def m_loader(ct):
            return xm[ct]

        xt2w = layernorm_wide("l2", m_loader, g2s, be2s, xt2_pool, "x2",
                              pools=(pj_ps, ln2_sq, ln2_tm, ln2_ve))
        ln2_stack.close()
        xt2 = {0: [t[:, 0:NH] for t in xt2w], 1: [t[:, NH:TOK] for t in xt2w]}
        for ct in range(CT):
            nc.vector.tensor_scalar_add(
                out=xm[ct],
                in0=xm[ct].bitcast(FP32),
                scalar1=b2s[:, ct:ct + 1])

        # ================= MLP =================
        with ExitStack() as ph:
            w1_pool = ph.enter_context(tc.tile_pool(name="w1s", bufs=6))
            w2_pool = ph.enter_context(tc.tile_pool(name="w2s", bufs=CH + 1))
            h1_pool = ph.enter_context(
                tc.tile_pool(name="h1", bufs=2 * CH + 2))
            mlp_ps = ph.enter_context(
                tc.tile_pool(name="mlpps", bufs=8, space="PSUM"))
            for fc in range(FT // CH):
                h1 = {}
                for half in range(CH // 4):
                    f0 = fc * CH + half * 4
                    psh = {}
                    for fi in range(4):
                        for nh in range(2):
                            psh[(fi, nh)] = mlp_ps.tile(
                                [128, NH], FP32, tag="mlp",
                                name=f"psh{fc}_{half}_{fi}_{nh}")
                    for ct in range(CT):
                        wt = w1_pool.tile([128, 512], BF16, tag="w1")
                        nc.gpsimd.dma_start(
                            out=wt,
                            in_=w1T[ct * 128:(ct + 1) * 128,
                                    f0 * 128:(f0 + 4) * 128])
                        for fi in range(4):
                            for nh in range(2):
                                nc.tensor.matmul(
                                    out=psh[(fi, nh)],
                                    lhsT=wt[:, fi * 128:(fi + 1) * 128],
                                    rhs=xt2[nh][ct],
                                    start=(ct == 0), stop=(ct == CT - 1))
                    for fi in range(4):
                        f = f0 + fi
                        for nh in range(2):
                            ht = h1_pool.tile([128, NH], BF16, tag="h1")
                            nc.scalar.activation(out=ht, in_=psh[(fi, nh)],
                                                 func=AF.Gelu,
                                                 bias=b1s[:, f:f + 1])
                            h1[(nh, half * 4 + fi)] = ht
                for qd in range(4):
                    w2ts = []
                    for fi in range(CH):
                        f = fc * CH + fi
                        wt = w2_pool.tile([128, 512], BF16, tag="w2")
                        nc.gpsimd.dma_start(
                            out=wt,
                            in_=w2T[f * 128:(f + 1) * 128,
                                    qd * 512:(qd + 1) * 512])
                        w2ts.append(wt)
                    for nh in range(2):
                        c0 = nh * NH
                        pss = [mlp_ps.tile([128, NH], FP32, tag="mlp",
                                           name=f"psw2_{fc}_{qd}_{nh}_{i}")
                               for i in range(4)]
                        for fi in range(CH):
                            for ot in range(4):
                                nc.tensor.matmul(
                                    out=pss[ot],
                                    lhsT=w2ts[fi][:, ot * 128:(ot + 1) * 128],
                                    rhs=h1[(nh, fi)],
                                    start=(fi == 0), stop=(fi == CH - 1))
                        for ot in range(4):
                            go = qd * 4 + ot
                            nc.vector.tensor_add(
                                out=xm[go][:, c0:c0 + NH],
                                in0=xm[go][:, c0:c0 + NH].bitcast(FP32),
                                in1=pss[ot])
        xt2_stack.close()

        # ================= Output =================
        for ct in range(CT):
            eng = nc.sync if ct % 2 == 0 else nc.gpsimd
            eng.dma_start(
                out=outT[ct * 128:(ct + 1) * 128, :],
                in_=xm[ct].bitcast(FP32))
        xmid_stack.close()

    _orig_tables = _patch_act_tables()
    try:
        nc.compile()
    finally:
        import concourse.bacc as _bacc
        _bacc.get_activation_tables = _orig_tables
    return nc


_NC = None


def _get_nc():
    global _NC
    if _NC is None:
        _NC = build()
    return _NC


def _to_fp8(a, scale=1.0):
    import ml_dtypes
    a = np.asarray(a, dtype=np.float32) * scale
    return np.ascontiguousarray(
        np.clip(a, -240.0, 240.0).astype(ml_dtypes.float8_e4m3))


def _dr_planes(wT, scale):
    """[DIM, 2048-out] weight (contraction-major) -> fp8 [128, 4*CT*512]
    laid out [c_inner(128), og(4), ct(16), oc(512)]."""
    w = np.asarray(wT, dtype=np.float32)
    w = w.reshape(CT, 128, 4, 512).transpose(1, 2, 0, 3).reshape(128, -1)
    return _to_fp8(w, scale)


def _prep_shared(Wqkv, Wproj, gamma1, beta1, gamma2, beta2, W1, b1, W2, b2):
    import ml_dtypes

    def f32(a):
        return np.ascontiguousarray(np.asarray(a, dtype=np.float32))

    def bf16(a):
        return np.ascontiguousarray(
            np.asarray(a, dtype=np.float32).astype(ml_dtypes.bfloat16))

    Wqkv = np.asarray(Wqkv)
    return {
        "wq8": _dr_planes(Wqkv[0:DIM].T, WS),
        "wk8": _dr_planes(Wqkv[DIM:2 * DIM].T, WS),
        "wv8": _dr_planes(Wqkv[2 * DIM:3 * DIM].T, VS),
        "wp8": _dr_planes(np.asarray(Wproj).T, WS),
        "w1T": bf16(np.asarray(W1).T),
        "w2T": bf16(np.asarray(W2).T),
        "g1v": f32(np.asarray(gamma1).reshape(CT, 128).T),
        "be1v": f32(np.asarray(beta1).reshape(CT, 128).T),
        "g2v": f32(np.asarray(gamma2).reshape(CT, 128).T),
        "be2v": f32(np.asarray(beta2).reshape(CT, 128).T),
        "b1v": f32(np.asarray(b1).reshape(FT, 128).T),
        "b2v": f32(np.asarray(b2).reshape(CT, 128).T),
    }


def build_in_maps(x, gamma1, beta1, Wqkv, Wproj, bproj, gamma2, beta2, W1,
                  b1, W2, b2):
    x = np.asarray(x, dtype=np.float32)          # [8, 2, 512, 2048]
    bproj = np.asarray(bproj, dtype=np.float32)
    shared = _prep_shared(Wqkv, Wproj, gamma1, beta1, gamma2, beta2,
                          W1, b1, W2, b2)
    in_maps = []
    for i in range(N_CORES):
        xt = np.ascontiguousarray(x[i].reshape(TOK, DIM).T)
        x8 = np.ascontiguousarray(
            xt.reshape(CT, 128, TOK).transpose(1, 0, 2))
        m = {"xT": np.ascontiguousarray(xt + bproj[:, None]),
             "x8": _to_fp8(x8)}
        m.update(shared)
        in_maps.append(m)
    return in_maps


def kernel(x, gamma1, beta1, Wqkv, Wproj, bproj, gamma2, beta2, W1, b1, W2,
           b2):
    nc = _get_nc()
    in_maps = build_in_maps(x, gamma1, beta1, Wqkv, Wproj, bproj, gamma2,
                            beta2, W1, b1, W2, b2)
    res = run_bass_kernel_spmd(nc, in_maps, core_ids=list(range(N_CORES)))
    out = np.stack([
        np.ascontiguousarray(res.results[i]["out"].T).reshape(2, NH, DIM)
        for i in range(N_CORES)
    ])
    return out
